# revision 19
# baseline (speedup 1.0000x reference)
"""Trainium2 Bass kernel for batched multi-head graph attention (GAT).

Reference computation (per batch b, head h):
    h_prime = h[b] @ w[h]                      # [N, FOUT]
    t = tanh(h_prime)
    src = t @ a_src[h]; dst = t @ a_dst[h]     # [N]
    s[i,j] = leaky_relu_{0.2}(src[i] + dst[j])
    attn = softmax_j(where(adj[b]>0, s, -inf))
    out[b,h] = attn @ h_prime

Device algorithm (core c <-> batch b=c):
    exp(leaky_relu(s)) = max(e^s, e^{0.2 s}); with s = src_i + dst_j the
    unnormalized weight factors as
        w[j,i] = adjT[j,i] * e^{src_i} * max(u_i * vq_j, q_j)
    with u = e^{-0.8 src}, vq = e^{0.2 dst}, q = e^{dst}. e^{src_i} cancels
    in the softmax ratio and is never computed; q is folded into the DVE
    tensor_scalar (mult, max with per-partition scalars), so the matmul
    stationary is just [h_prime | 1] and row 64 accumulates the softmax
    denominator.

    Projections run on the PE: head pairs packed as [128,128] stationaries
    give tanh(h_prime)^T in [f, n] layout; small matmuls against a
    block-diagonal [128, 4] projection produce src/dst directly in
    j-partition column form (sdc).  src columns are PE-transposed back to
    rows, exponentiated, bounced through DRAM with a stride-0 broadcast DMA
    to build the 128-partition u rows.  Head 7's DVE work runs on GpSimd.

    The host divides rows 0..63 by row 64 and transposes to [b, h, n, f].
"""

import numpy as np
import ml_dtypes

import concourse.mybir as mybir
import concourse.tile as tile
from concourse import bacc
from concourse.bass_utils import run_bass_kernel_spmd

BS, N, FIN, NH, FOUT = 8, 1024, 256, 8, 64
P = 128
NCH = N // P          # 8 chunks of the node axis
KC = FIN // P         # 2 chunks of the feature-in axis
NPAIR = NH // 2       # head pairs for the packed projection matmuls
F32 = mybir.dt.float32
BF16 = mybir.dt.bfloat16
ALU = mybir.AluOpType
ACTF = mybir.ActivationFunctionType
BF16NP = ml_dtypes.bfloat16

HLF = 512  # moving-operand half width (PSUM bank limit at fp32 out)
GPSIMD_HEADS = ()  # gpsimd TS measured 14.9us/op on HW -- keep all DVE


def emit(nc, tc, hT_d, w_d, a4_d, adjT_d, ident_d, uscr_d, out_d):
    with (
        tc.tile_pool(name="const", bufs=1) as cpool,
        tc.tile_pool(name="t2", bufs=3) as tpool,
        tc.tile_pool(name="mx", bufs=4) as mxpool,
        tc.tile_pool(name="z", bufs=4) as zpool,
        tc.tile_pool(name="osb", bufs=2) as opool,
    ):
        # ---- constant tiles ----
        hT = cpool.tile([P, KC, N], BF16)
        wsb = cpool.tile([P, KC, NH * FOUT], BF16)
        a4 = cpool.tile([P, NPAIR, 4], BF16)
        adjT = cpool.tile([P, NCH, N], BF16)
        ident = cpool.tile([P, P], F32)
        warm_src = cpool.tile([P, HLF], BF16)
        nc.vector.memset(warm_src, 0.0)

        hp_tiles = [cpool.tile([P, NH, 66], BF16, name=f"hp{ic}") for ic in range(NCH)]
        for ic in range(NCH):
            nc.vector.memset(hp_tiles[ic][:, :, 64:66], 1.0)

        scolb = cpool.tile([P, NCH, NPAIR, 2], F32)
        q_col = cpool.tile([P, NCH, NH], F32)
        vq_col = cpool.tile([P, NCH, NH], F32)
        u_p = [cpool.tile([2, N], BF16, name=f"u{p}") for p in range(NPAIR)]
        ub_tiles = [cpool.tile([P, N], BF16, name=f"ub{h}") for h in range(NH)]

        # ---- input DMAs: one queue, ordered so phase A unblocks first and
        # the big adjacency matrix streams in behind it ----
        nc.sync.dma_start(wsb, w_d.rearrange("kc p f -> p kc f"))
        for kc in range(KC):
            nc.sync.dma_start(hT[:, kc, :], hT_d[kc])
        nc.sync.dma_start(ident, ident_d)
        nc.sync.dma_start(a4, a4_d.rearrange("q p f -> p q f"))
        # adjT is not needed until the first TT (~15us after phase A starts);
        # putting it last on the same queue keeps HBM free for hT/wsb.
        nc.sync.dma_start(adjT, adjT_d.rearrange("jc p n -> p jc n"))

        with (
            tc.tile_pool(name="psa1", bufs=2, space="PSUM") as pp_a1,
            tc.tile_pool(name="psa2", bufs=2, space="PSUM") as pp_a2,
            tc.tile_pool(name="psdc", bufs=1, space="PSUM") as pp_dc,
            tc.tile_pool(name="psut", bufs=2, space="PSUM") as pp_ut,
        ):
            # ---- PE warm-up during the input-DMA window (HAM to K=8/8) ----
            warm = pp_a1.tile([P, HLF], F32, tag="a1", name="warm")
            for i in range(6):
                nc.tensor.matmul(
                    warm, warm_src[:, 0:P], warm_src, start=True, stop=True
                )

            sdc = pp_dc.tile([P, NCH, NPAIR, 4], F32)

            # ---- phase A2: tanh(h_prime)^T per pair; sdc projections ----
            for pr in range(NPAIR):
                t2 = tpool.tile([P, N], BF16)
                for hf in range(2):
                    ps2 = pp_a2.tile([P, HLF], F32, tag="a2")
                    for kc in range(KC):
                        nc.tensor.matmul(
                            ps2,
                            wsb[:, kc, 2 * pr * FOUT : (2 * pr + 2) * FOUT],
                            hT[:, kc, hf * HLF : (hf + 1) * HLF],
                            start=(kc == 0),
                            stop=(kc == KC - 1),
                        )
                    nc.scalar.activation(
                        t2[:, hf * HLF : (hf + 1) * HLF], ps2, ACTF.Tanh
                    )
                for ic in range(NCH):
                    nc.tensor.matmul(
                        sdc[:, ic, pr, :],
                        t2[:, ic * P : (ic + 1) * P],
                        a4[:, pr, :],
                        start=True,
                        stop=True,
                    )
                # dst columns -> per-partition scalars
                nc.scalar.activation(
                    q_col[:, :, 2 * pr : 2 * pr + 2], sdc[:, :, pr, 2:4], ACTF.Exp
                )
                nc.scalar.activation(
                    vq_col[:, :, 2 * pr : 2 * pr + 2],
                    sdc[:, :, pr, 2:4],
                    ACTF.Exp,
                    scale=0.2,
                )
                # src columns -> SBUF -> transpose back to rows -> exp -> DRAM
                nc.scalar.activation(
                    scolb[:, :, pr, :], sdc[:, :, pr, 0:2], ACTF.Copy
                )
                for hf in range(2):
                    ut = pp_ut.tile([2, HLF], F32, tag="ut")
                    for i4 in range(4):
                        ic = hf * 4 + i4
                        nc.tensor.transpose(
                            ut[:, i4 * P : (i4 + 1) * P], scolb[:, ic, pr, :], ident
                        )
                    nc.scalar.activation(
                        u_p[pr][:, hf * HLF : (hf + 1) * HLF],
                        ut,
                        ACTF.Exp,
                        scale=-0.8,
                    )
                # scalar's DGE queue is idle and, unlike sync's, not backed up
                # behind the 2MB adjT stream; same-queue FIFO orders store
                # before the broadcast reads.
                nc.scalar.dma_start(uscr_d[pr], u_p[pr])
                for k in range(2):
                    nc.scalar.dma_start(
                        ub_tiles[2 * pr + k],
                        uscr_d[pr, k][None, :].to_broadcast([P, N]),
                    )

            # ---- phase A1: h_prime in [n, h*f] layout for the stationaries ----
            for ic in range(NCH):
                ps1 = pp_a1.tile([P, NH * FOUT], F32, tag="a1")
                for kc in range(KC):
                    nc.tensor.matmul(
                        ps1,
                        hT[:, kc, ic * P : (ic + 1) * P],
                        wsb[:, kc, :],
                        start=(kc == 0),
                        stop=(kc == KC - 1),
                    )
                nc.scalar.activation(
                    hp_tiles[ic][:, :, 0:FOUT],
                    ps1.rearrange("p (h f) -> p h f", f=FOUT),
                    ACTF.Copy,
                )

        # ---- phase C: masked weights + attention matmuls ----
        def emit_mxz(eng, h):
            """TS (mult,max) per j-chunk + one TT mask-mult per 2 chunks."""
            zs = []
            for g in range(4):
                mx = mxpool.tile([P, 2, N], BF16, tag="mx", name=f"mx{h}_{g}")
                for k in range(2):
                    jc = 2 * g + k
                    eng.tensor_scalar(
                        mx[:, k, :],
                        ub_tiles[h],
                        vq_col[:, jc, h : h + 1],
                        q_col[:, jc, h : h + 1],
                        ALU.mult,
                        ALU.max,
                    )
                z = zpool.tile([P, 2, N], BF16, tag="z", name=f"z{h}_{g}")
                eng.tensor_tensor(z, mx, adjT[:, 2 * g : 2 * g + 2, :], ALU.mult)
                zs.append(z)
            return zs

        # gpsimd heads start as soon as their inputs exist, in parallel
        gp_z = {h: emit_mxz(nc.gpsimd, h) for h in GPSIMD_HEADS}

        with tc.tile_pool(name="psout", bufs=4, space="PSUM") as pp_out:
            for h in range(NH):
                zs = gp_z[h] if h in GPSIMD_HEADS else emit_mxz(nc.vector, h)
                pso = [
                    pp_out.tile([FOUT + 1, HLF], F32, tag="out", name=f"o{h}_{hf}")
                    for hf in range(2)
                ]
                for jc in range(NCH):
                    for hf in range(2):
                        nc.tensor.matmul(
                            pso[hf],
                            hp_tiles[jc][:, h, 0:65],
                            zs[jc // 2][:, jc % 2, hf * HLF : (hf + 1) * HLF],
                            start=(jc == 0),
                            stop=(jc == NCH - 1),
                        )
                ot = opool.tile([FOUT + 1, N], F32)
                for hf in range(2):
                    nc.scalar.activation(
                        ot[:, hf * HLF : (hf + 1) * HLF], pso[hf], ACTF.Copy
                    )
                    nc.sync.dma_start(
                        out_d[h, :, hf * HLF : (hf + 1) * HLF],
                        ot[:, hf * HLF : (hf + 1) * HLF],
                    )


def build_program(num_devices=8, debug=False):
    nc = bacc.Bacc(
        "TRN2", target_bir_lowering=False, debug=debug, num_devices=num_devices
    )
    hT_d = nc.dram_tensor("hT", [KC, P, N], BF16, kind="ExternalInput").ap()
    w_d = nc.dram_tensor("w_all", [KC, P, NH * FOUT], BF16, kind="ExternalInput").ap()
    a4_d = nc.dram_tensor("a4", [NPAIR, P, 4], BF16, kind="ExternalInput").ap()
    adjT_d = nc.dram_tensor("adjT", [NCH, P, N], BF16, kind="ExternalInput").ap()
    ident_d = nc.dram_tensor("ident", [P, P], F32, kind="ExternalInput").ap()
    uscr_d = nc.dram_tensor("uscr", [NPAIR, 2, N], BF16, kind="Internal").ap()
    out_d = nc.dram_tensor("outT", [NH, FOUT + 1, N], F32, kind="ExternalOutput").ap()
    with tile.TileContext(nc) as tc:
        emit(nc, tc, hT_d, w_d, a4_d, adjT_d, ident_d, uscr_d, out_d)
    nc.compile()
    return nc


def make_in_maps(h, adj, w, a_src, a_dst):
    """Host-side sharding/layout prep: core c gets batch c."""
    w_all = np.ascontiguousarray(
        w.astype(np.float32).transpose(1, 0, 2).reshape(KC, P, NH * FOUT)
    ).astype(BF16NP)
    a4 = np.zeros((NPAIR, P, 4), dtype=np.float32)
    for pr in range(NPAIR):
        a4[pr, 0:FOUT, 0] = a_src[2 * pr, :, 0]
        a4[pr, FOUT:P, 1] = a_src[2 * pr + 1, :, 0]
        a4[pr, 0:FOUT, 2] = a_dst[2 * pr, :, 0]
        a4[pr, FOUT:P, 3] = a_dst[2 * pr + 1, :, 0]
    a4 = a4.astype(BF16NP)
    ident = np.eye(P, dtype=np.float32)
    in_maps = []
    for b in range(BS):
        hT = np.ascontiguousarray(
            h[b].astype(np.float32).T.reshape(KC, P, N)
        ).astype(BF16NP)
        adjT = np.ascontiguousarray(adj[b].T.reshape(NCH, P, N)).astype(BF16NP)
        in_maps.append(
            {"hT": hT, "w_all": w_all, "a4": a4, "adjT": adjT, "ident": ident}
        )
    return in_maps


def postprocess(raw_outs):
    """raw_outs: list of [NH, FOUT+1, N] per core -> full [BS, NH, N, FOUT]."""
    outT = np.stack(raw_outs)  # [BS, NH, FOUT+1, N]
    num = outT[:, :, 0:FOUT, :]
    den = outT[:, :, FOUT : FOUT + 1, :]
    return np.ascontiguousarray((num / den).transpose(0, 1, 3, 2)).astype(np.float32)


_NC_CACHE = {}


def kernel(h, adj, w, a_src, a_dst):
    if "nc" not in _NC_CACHE:
        _NC_CACHE["nc"] = build_program(num_devices=BS)
    nc = _NC_CACHE["nc"]
    in_maps = make_in_maps(h, adj, w, a_src, a_dst)
    res = run_bass_kernel_spmd(nc, in_maps, core_ids=list(range(BS)))
    return postprocess([r["outT"] for r in res.results])


# revision 20
# speedup vs baseline: 1.1785x; 1.1785x over previous
"""Trainium2 Bass kernel for batched multi-head graph attention (GAT).

Reference computation (per batch b, head h):
    h_prime = h[b] @ w[h]                      # [N, FOUT]
    t = tanh(h_prime)
    src = t @ a_src[h]; dst = t @ a_dst[h]     # [N]
    s[i,j] = leaky_relu_{0.2}(src[i] + dst[j])
    attn = softmax_j(where(adj[b]>0, s, -inf))
    out[b,h] = attn @ h_prime

Device algorithm (core c <-> batch b=c):
    exp(leaky_relu(s)) = max(e^s, e^{0.2 s}); with s = src_i + dst_j the
    unnormalized weight factors as
        w[j,i] = adjT[j,i] * e^{src_i} * max(u_i * vq_j, q_j)
    with u = e^{-0.8 src}, vq = e^{0.2 dst}, q = e^{dst}. e^{src_i} cancels
    in the softmax ratio and is never computed; q is folded into the DVE
    tensor_scalar (mult, max with per-partition scalars), so the matmul
    stationary is just [h_prime | 1] and row 64 accumulates the softmax
    denominator.

    Projections run on the PE: head pairs packed as [128,128] stationaries
    give tanh(h_prime)^T in [f, n] layout; small matmuls against a
    block-diagonal [128, 4] projection produce src/dst directly in
    j-partition column form (sdc).  src columns are PE-transposed back to
    rows, exponentiated, bounced through DRAM with a stride-0 broadcast DMA
    to build the 128-partition u rows.  Head 7's DVE work runs on GpSimd.

    The host divides rows 0..63 by row 64 and transposes to [b, h, n, f].
"""

import numpy as np
import ml_dtypes

import concourse.mybir as mybir
import concourse.tile as tile
from concourse import bacc
from concourse.bass_utils import run_bass_kernel_spmd

BS, N, FIN, NH, FOUT = 8, 1024, 256, 8, 64
P = 128
NCH = N // P          # 8 chunks of the node axis
KC = FIN // P         # 2 chunks of the feature-in axis
NPAIR = NH // 2       # head pairs for the packed projection matmuls
F32 = mybir.dt.float32
BF16 = mybir.dt.bfloat16
ALU = mybir.AluOpType
ACTF = mybir.ActivationFunctionType
BF16NP = ml_dtypes.bfloat16

HLF = 512  # moving-operand half width (PSUM bank limit at fp32 out)
GPSIMD_HEADS = ()  # gpsimd TS measured 14.9us/op on HW -- keep all DVE


def emit(nc, tc, hT_d, w_d, a4_d, adjT_d, ident_d, uscr_d, out_d):
    with (
        tc.tile_pool(name="const", bufs=1) as cpool,
        tc.tile_pool(name="t2", bufs=3) as tpool,
        tc.tile_pool(name="mx", bufs=4) as mxpool,
        tc.tile_pool(name="z", bufs=4) as zpool,
        tc.tile_pool(name="osb", bufs=2) as opool,
    ):
        # ---- constant tiles ----
        hT = cpool.tile([P, KC, N], BF16)
        wsb = cpool.tile([P, KC, NH * FOUT], BF16)
        a4 = cpool.tile([P, NPAIR, 4], BF16)
        adjT = cpool.tile([P, NCH, N], BF16)
        ident = cpool.tile([P, P], F32)
        warm_src = cpool.tile([P, HLF], BF16)
        nc.vector.memset(warm_src, 0.0)

        hp_tiles = [cpool.tile([P, NH, 66], BF16, name=f"hp{ic}") for ic in range(NCH)]
        for ic in range(NCH):
            nc.vector.memset(hp_tiles[ic][:, :, 64:66], 1.0)

        scolb = cpool.tile([P, NCH, NPAIR, 2], F32)
        q_col = cpool.tile([P, NCH, NH], F32)
        vq_col = cpool.tile([P, NCH, NH], F32)
        u_p = [cpool.tile([2, N], BF16, name=f"u{p}") for p in range(NPAIR)]
        ub_tiles = [cpool.tile([P, N], BF16, name=f"ub{h}") for h in range(NH)]

        # ---- input DMAs: one queue, ordered so phase A unblocks first and
        # the big adjacency matrix streams in behind it ----
        nc.sync.dma_start(wsb, w_d.rearrange("kc p f -> p kc f"))
        for kc in range(KC):
            nc.sync.dma_start(hT[:, kc, :], hT_d[kc])
        nc.sync.dma_start(ident, ident_d)
        nc.sync.dma_start(a4, a4_d.rearrange("q p f -> p q f"))
        # adjT is not needed until the first TT (~15us after phase A starts);
        # putting it last on the same queue keeps HBM free for hT/wsb.
        nc.sync.dma_start(adjT, adjT_d.rearrange("jc p n -> p jc n"))

        with (
            tc.tile_pool(name="psa1", bufs=2, space="PSUM") as pp_a1,
            tc.tile_pool(name="psa2", bufs=2, space="PSUM") as pp_a2,
            tc.tile_pool(name="psdc", bufs=1, space="PSUM") as pp_dc,
            tc.tile_pool(name="psut", bufs=2, space="PSUM") as pp_ut,
        ):
            # ---- PE warm-up during the input-DMA window (HAM to K=8/8) ----
            warm = pp_a1.tile([P, HLF], F32, tag="a1", name="warm")
            for i in range(6):
                nc.tensor.matmul(
                    warm, warm_src[:, 0:P], warm_src, start=True, stop=True
                )

            sdc = pp_dc.tile([P, NCH, NPAIR, 4], F32)

            # ---- phase A2: tanh(h_prime)^T per pair; sdc projections ----
            for pr in range(NPAIR):
                t2 = tpool.tile([P, N], BF16)
                for hf in range(2):
                    ps2 = pp_a2.tile([P, HLF], F32, tag="a2")
                    for kc in range(KC):
                        nc.tensor.matmul(
                            ps2,
                            wsb[:, kc, 2 * pr * FOUT : (2 * pr + 2) * FOUT],
                            hT[:, kc, hf * HLF : (hf + 1) * HLF],
                            start=(kc == 0),
                            stop=(kc == KC - 1),
                        )
                    nc.scalar.activation(
                        t2[:, hf * HLF : (hf + 1) * HLF], ps2, ACTF.Tanh
                    )
                for ic in range(NCH):
                    nc.tensor.matmul(
                        sdc[:, ic, pr, :],
                        t2[:, ic * P : (ic + 1) * P],
                        a4[:, pr, :],
                        start=True,
                        stop=True,
                    )
                # dst columns -> per-partition scalars
                nc.scalar.activation(
                    q_col[:, :, 2 * pr : 2 * pr + 2], sdc[:, :, pr, 2:4], ACTF.Exp
                )
                nc.scalar.activation(
                    vq_col[:, :, 2 * pr : 2 * pr + 2],
                    sdc[:, :, pr, 2:4],
                    ACTF.Exp,
                    scale=0.2,
                )
                # src columns -> SBUF -> transpose back to rows -> exp -> DRAM
                nc.scalar.activation(
                    scolb[:, :, pr, :], sdc[:, :, pr, 0:2], ACTF.Copy
                )
                for hf in range(2):
                    ut = pp_ut.tile([2, HLF], F32, tag="ut")
                    for i4 in range(4):
                        ic = hf * 4 + i4
                        nc.tensor.transpose(
                            ut[:, i4 * P : (i4 + 1) * P], scolb[:, ic, pr, :], ident
                        )
                    nc.scalar.activation(
                        u_p[pr][:, hf * HLF : (hf + 1) * HLF],
                        ut,
                        ACTF.Exp,
                        scale=-0.8,
                    )
                # same-queue FIFO (sync) orders the store before the
                # broadcast reads; issuing from scalar's DGE instead delays
                # the critical activation chain (measured +18us).
                nc.sync.dma_start(uscr_d[pr], u_p[pr])
                for k in range(2):
                    nc.sync.dma_start(
                        ub_tiles[2 * pr + k],
                        uscr_d[pr, k][None, :].to_broadcast([P, N]),
                    )

            # ---- phase A1: h_prime in [n, h*f] layout for the stationaries ----
            for ic in range(NCH):
                ps1 = pp_a1.tile([P, NH * FOUT], F32, tag="a1")
                for kc in range(KC):
                    nc.tensor.matmul(
                        ps1,
                        hT[:, kc, ic * P : (ic + 1) * P],
                        wsb[:, kc, :],
                        start=(kc == 0),
                        stop=(kc == KC - 1),
                    )
                nc.scalar.activation(
                    hp_tiles[ic][:, :, 0:FOUT],
                    ps1.rearrange("p (h f) -> p h f", f=FOUT),
                    ACTF.Copy,
                )

        # ---- phase C: masked weights + attention matmuls ----
        def emit_mxz(eng, h):
            """TS (mult,max) per j-chunk + one TT mask-mult per 2 chunks."""
            zs = []
            for g in range(4):
                mx = mxpool.tile([P, 2, N], BF16, tag="mx", name=f"mx{h}_{g}")
                for k in range(2):
                    jc = 2 * g + k
                    eng.tensor_scalar(
                        mx[:, k, :],
                        ub_tiles[h],
                        vq_col[:, jc, h : h + 1],
                        q_col[:, jc, h : h + 1],
                        ALU.mult,
                        ALU.max,
                    )
                z = zpool.tile([P, 2, N], BF16, tag="z", name=f"z{h}_{g}")
                eng.tensor_tensor(z, mx, adjT[:, 2 * g : 2 * g + 2, :], ALU.mult)
                zs.append(z)
            return zs

        # gpsimd heads start as soon as their inputs exist, in parallel
        gp_z = {h: emit_mxz(nc.gpsimd, h) for h in GPSIMD_HEADS}

        with tc.tile_pool(name="psout", bufs=4, space="PSUM") as pp_out:
            for h in range(NH):
                zs = gp_z[h] if h in GPSIMD_HEADS else emit_mxz(nc.vector, h)
                pso = [
                    pp_out.tile([FOUT + 1, HLF], F32, tag="out", name=f"o{h}_{hf}")
                    for hf in range(2)
                ]
                for jc in range(NCH):
                    for hf in range(2):
                        nc.tensor.matmul(
                            pso[hf],
                            hp_tiles[jc][:, h, 0:65],
                            zs[jc // 2][:, jc % 2, hf * HLF : (hf + 1) * HLF],
                            start=(jc == 0),
                            stop=(jc == NCH - 1),
                        )
                ot = opool.tile([FOUT + 1, N], F32)
                for hf in range(2):
                    nc.scalar.activation(
                        ot[:, hf * HLF : (hf + 1) * HLF], pso[hf], ACTF.Copy
                    )
                    nc.sync.dma_start(
                        out_d[h, :, hf * HLF : (hf + 1) * HLF],
                        ot[:, hf * HLF : (hf + 1) * HLF],
                    )


def build_program(num_devices=8, debug=False):
    nc = bacc.Bacc(
        "TRN2", target_bir_lowering=False, debug=debug, num_devices=num_devices
    )
    hT_d = nc.dram_tensor("hT", [KC, P, N], BF16, kind="ExternalInput").ap()
    w_d = nc.dram_tensor("w_all", [KC, P, NH * FOUT], BF16, kind="ExternalInput").ap()
    a4_d = nc.dram_tensor("a4", [NPAIR, P, 4], BF16, kind="ExternalInput").ap()
    adjT_d = nc.dram_tensor("adjT", [NCH, P, N], BF16, kind="ExternalInput").ap()
    ident_d = nc.dram_tensor("ident", [P, P], F32, kind="ExternalInput").ap()
    uscr_d = nc.dram_tensor("uscr", [NPAIR, 2, N], BF16, kind="Internal").ap()
    out_d = nc.dram_tensor("outT", [NH, FOUT + 1, N], F32, kind="ExternalOutput").ap()
    with tile.TileContext(nc) as tc:
        emit(nc, tc, hT_d, w_d, a4_d, adjT_d, ident_d, uscr_d, out_d)
    nc.compile()
    return nc


def make_in_maps(h, adj, w, a_src, a_dst):
    """Host-side sharding/layout prep: core c gets batch c."""
    w_all = np.ascontiguousarray(
        w.astype(np.float32).transpose(1, 0, 2).reshape(KC, P, NH * FOUT)
    ).astype(BF16NP)
    a4 = np.zeros((NPAIR, P, 4), dtype=np.float32)
    for pr in range(NPAIR):
        a4[pr, 0:FOUT, 0] = a_src[2 * pr, :, 0]
        a4[pr, FOUT:P, 1] = a_src[2 * pr + 1, :, 0]
        a4[pr, 0:FOUT, 2] = a_dst[2 * pr, :, 0]
        a4[pr, FOUT:P, 3] = a_dst[2 * pr + 1, :, 0]
    a4 = a4.astype(BF16NP)
    ident = np.eye(P, dtype=np.float32)
    in_maps = []
    for b in range(BS):
        hT = np.ascontiguousarray(
            h[b].astype(np.float32).T.reshape(KC, P, N)
        ).astype(BF16NP)
        adjT = np.ascontiguousarray(adj[b].T.reshape(NCH, P, N)).astype(BF16NP)
        in_maps.append(
            {"hT": hT, "w_all": w_all, "a4": a4, "adjT": adjT, "ident": ident}
        )
    return in_maps


def postprocess(raw_outs):
    """raw_outs: list of [NH, FOUT+1, N] per core -> full [BS, NH, N, FOUT]."""
    outT = np.stack(raw_outs)  # [BS, NH, FOUT+1, N]
    num = outT[:, :, 0:FOUT, :]
    den = outT[:, :, FOUT : FOUT + 1, :]
    return np.ascontiguousarray((num / den).transpose(0, 1, 3, 2)).astype(np.float32)


_NC_CACHE = {}


def kernel(h, adj, w, a_src, a_dst):
    if "nc" not in _NC_CACHE:
        _NC_CACHE["nc"] = build_program(num_devices=BS)
    nc = _NC_CACHE["nc"]
    in_maps = make_in_maps(h, adj, w, a_src, a_dst)
    res = run_bass_kernel_spmd(nc, in_maps, core_ids=list(range(BS)))
    return postprocess([r["outT"] for r in res.results])
